# revision 8
# baseline (speedup 1.0000x reference)
"""DINet retrieval-knn kernel for 8 trn2 NeuronCores.

Math (see reference): for each query patch q (3x3xC neighborhood of Q),
find k* = argmax_k cos(K_patch_k, Q_patch_q) over all 4096 key patches,
output S = max cosine value, T = fold(V_patch_gather(k*)) / 9.

Device strategy (per sharding hint): data-parallel over batch B (=2),
sequence-parallel over Q columns (4 shards of 1024) -> 8 cores. Each core
computes its full [Lk=4096, Lq=1024] correlation block with the tensor
engine (contraction C*9=576 in fp32), and a fused
copy+max (tensor_tensor_reduce) plus max_index pass gives max/argmax over
the full K axis per query. Host does layout prep (unfold, l2-normalize)
and the final V-gather + fold.
"""

import sys

import numpy as np

for _p in ("/opt/trn_rl_repo", "/root/.axon_site/_ro/trn_rl_repo"):
    if _p not in sys.path:
        sys.path.append(_p)

import concourse.bass as bass
import concourse.mybir as mybir
from concourse import bacc, bass_utils
from concourse.tile import TileContext

B, C, H, W = 2, 64, 64, 64
L = H * W            # 4096
C9 = C * 9           # 576
NSHARD = 4           # Q-column shards per batch
LQ = L // NSHARD     # 1024 query columns per core
NCORES = 8
NQB = LQ // 128      # 8 query blocks of 128
NKT = L // 512       # 8 key column tiles of 512
# contraction chunks over C9=576: rows (start, size)
CHUNKS = [(0, 128), (128, 128), (256, 128), (384, 128), (512, 64)]

EPS = 1e-12

_BASS_CACHE = {}


def _build_bass():
    f32 = mybir.dt.float32
    u32 = mybir.dt.uint32
    # Bacc (not plain Bass): its compile() runs move_matmul_waits_to_ldweights
    # + generate_event_semaphores, which split multi-wait instructions that
    # walrus otherwise rejects ("Too many sync wait commands")
    nc = bacc.Bacc("TRN2")

    kn = nc.dram_tensor("kn", [C9, L], f32, kind="ExternalInput")
    qn = nc.dram_tensor("qn", [C9, LQ], f32, kind="ExternalInput")
    rmax = nc.dram_tensor("rmax", [128, NQB], f32, kind="ExternalOutput")
    rarg = nc.dram_tensor("rarg", [128, NQB], u32, kind="ExternalOutput")

    with TileContext(nc) as tc:
        with (
            tc.tile_pool(name="kpool", bufs=1) as kpool,
            tc.tile_pool(name="qpool", bufs=1) as qpool,
            tc.tile_pool(name="rpool", bufs=3) as rpool,
            tc.tile_pool(name="ppool", bufs=2, space="PSUM") as ppool,
            tc.tile_pool(name="mpool", bufs=2) as mpool,
            tc.tile_pool(name="opool", bufs=1) as opool,
        ):
            # persistent input tiles; small tiles so compute can start as
            # soon as the first slices land
            qt = [
                [
                    qpool.tile([p, 128], f32, name=f"qt{t}_{qb}", tag=f"qt{t}_{qb}")
                    for qb in range(NQB)
                ]
                for t, (r0, p) in enumerate(CHUNKS)
            ]
            kt = [
                [
                    kpool.tile([p, 512], f32, name=f"kt{t}_{n}", tag=f"kt{t}_{n}")
                    for n in range(NKT)
                ]
                for t, (r0, p) in enumerate(CHUNKS)
            ]
            # DMA issue order ~ consumption order
            for t, (r0, p) in enumerate(CHUNKS):
                nc.sync.dma_start(out=qt[t][0], in_=qn[r0 : r0 + p, 0:128])
            for n in range(NKT):
                for t, (r0, p) in enumerate(CHUNKS):
                    nc.sync.dma_start(
                        out=kt[t][n], in_=kn[r0 : r0 + p, n * 512 : (n + 1) * 512]
                    )
            for qb in range(1, NQB):
                for t, (r0, p) in enumerate(CHUNKS):
                    nc.sync.dma_start(
                        out=qt[t][qb], in_=qn[r0 : r0 + p, qb * 128 : (qb + 1) * 128]
                    )

            outv = opool.tile([128, NQB], f32, name="outv", tag="outv")
            outi = opool.tile([128, NQB], u32, name="outi", tag="outi")

            for qb in range(NQB):
                rt = rpool.tile([128, L], f32, name="rt", tag="rt")
                for kh in range(2):
                    ps = ppool.tile([128, 2048], f32, name="ps", tag="ps")
                    for t in range(5):
                        for n in range(4):
                            nc.tensor.matmul(
                                ps[:, n * 512 : (n + 1) * 512],
                                lhsT=qt[t][qb],
                                rhs=kt[t][kh * 4 + n],
                                start=(t == 0),
                                stop=(t == 4),
                            )
                    nc.scalar.copy(out=rt[:, kh * 2048 : (kh + 1) * 2048], in_=ps)
                nc.vector.reduce_max(
                    out=outv[:, qb : qb + 1], in_=rt, axis=mybir.AxisListType.X
                )
                mx8 = mpool.tile([128, 8], f32, name="mx8", tag="mx8")
                nc.vector.tensor_copy(
                    out=mx8, in_=outv[:, qb : qb + 1].to_broadcast([128, 8])
                )
                idx8 = mpool.tile([128, 8], u32, name="idx8", tag="idx8")
                nc.vector.max_index(out=idx8, in_max=mx8, in_values=rt)
                nc.gpsimd.tensor_copy(
                    out=outi[:, qb : qb + 1], in_=idx8[:, 0:1]
                )

            nc.sync.dma_start(out=rmax[:, :], in_=outv)
            nc.sync.dma_start(out=rarg[:, :], in_=outi)
    if not nc.is_finalized():
        nc.finalize()
    return nc


def _unfold_ij(x):
    """[B,C,H,W] -> [B, 9*C, H*W] with row = ij*C + c (ij-major order)."""
    b, c, h, w = x.shape
    xp = np.pad(x, ((0, 0), (0, 0), (1, 1), (1, 1)))
    blocks = [
        xp[:, :, i : i + h, j : j + w].reshape(b, c, h * w)
        for i in range(3)
        for j in range(3)
    ]
    return np.concatenate(blocks, axis=1)


def _unfold_torch(x):
    """[B,C,H,W] -> [B, C*9, H*W] in torch F.unfold order (c-major)."""
    b, c, h, w = x.shape
    xp = np.pad(x, ((0, 0), (0, 0), (1, 1), (1, 1)))
    patches = np.stack(
        [xp[:, :, i : i + h, j : j + w] for i in range(3) for j in range(3)],
        axis=2,
    )
    return patches.reshape(b, c * 9, h * w)


def _fold_torch(u, h, w):
    """Inverse layout of _unfold_torch: sum overlapping patches."""
    b, ck, l = u.shape
    c = ck // 9
    p = u.reshape(b, c, 3, 3, h, w)
    out = np.zeros((b, c, h + 2, w + 2), u.dtype)
    for i in range(3):
        for j in range(3):
            out[:, :, i : i + h, j : j + w] += p[:, :, i, j]
    return out[:, :, 1 : 1 + h, 1 : 1 + w]


def _l2n_cols(x):
    """Normalize columns of [B, C9, L] (fp32, eps as in reference)."""
    n = np.sqrt(np.sum(x * x, axis=1, keepdims=True, dtype=np.float32))
    return x / np.maximum(n, EPS)


def _run_device(Kn, Qn, trace=False, trace_cores=None):
    key = "nc"
    if key not in _BASS_CACHE:
        _BASS_CACHE[key] = _build_bass()
    nc = _BASS_CACHE[key]
    in_maps = []
    for ci in range(NCORES):
        b, s = divmod(ci, NSHARD)
        in_maps.append(
            {
                "kn": np.ascontiguousarray(Kn[b]),
                "qn": np.ascontiguousarray(Qn[b][:, s * LQ : (s + 1) * LQ]),
            }
        )
    res = bass_utils.run_bass_kernel_spmd(
        nc,
        in_maps,
        core_ids=list(range(NCORES)),
        trace=trace,
        trace_cores=trace_cores,
    )
    return res


def kernel(V, K, Q, _trace=False, _trace_cores=None, _return_results=False):
    V = np.asarray(V, dtype=np.float32)
    K = np.asarray(K, dtype=np.float32)
    Q = np.asarray(Q, dtype=np.float32)

    Kn = _l2n_cols(_unfold_ij(K))
    Qn = _l2n_cols(_unfold_ij(Q))

    res = _run_device(Kn, Qn, trace=_trace, trace_cores=_trace_cores)

    rstar = np.empty((B, L), np.float32)
    rarg = np.empty((B, L), np.int64)
    for ci in range(NCORES):
        b, s = divmod(ci, NSHARD)
        out = res.results[ci]
        # out[p, qb] -> local q index qb*128 + p
        rstar[b, s * LQ : (s + 1) * LQ] = (
            np.asarray(out["rmax"]).transpose(1, 0).reshape(-1)
        )
        rarg[b, s * LQ : (s + 1) * LQ] = (
            np.asarray(out["rarg"]).astype(np.int64).transpose(1, 0).reshape(-1)
        )

    V_unf = _unfold_torch(V)
    T_unf = np.take_along_axis(V_unf, rarg[:, None, :], axis=2)
    T = (_fold_torch(T_unf, H, W) / 9.0).astype(np.float32)
    S = rstar.reshape(B, 1, H, W)

    if _return_results:
        return (S, T), res
    return (S, T)


# revision 12
# speedup vs baseline: 2.4296x; 2.4296x over previous
"""DINet retrieval-knn kernel for 8 trn2 NeuronCores.

Math (see reference): for each query patch q (3x3xC neighborhood of Q),
find k* = argmax_k cos(K_patch_k, Q_patch_q) over all 4096 key patches,
output S = max cosine value, T = fold(V_patch_gather(k*)) / 9.

Device strategy (per sharding hint): data-parallel over batch B (=2),
sequence-parallel over Q columns (4 shards of 1024) -> 8 cores. Each core
computes its full [Lk=4096, Lq=1024] correlation block with the tensor
engine (contraction C*9=576 in fp32), and a fused
copy+max (tensor_tensor_reduce) plus max_index pass gives max/argmax over
the full K axis per query. Host does layout prep (unfold, l2-normalize)
and the final V-gather + fold.
"""

import sys

import numpy as np

for _p in ("/opt/trn_rl_repo", "/root/.axon_site/_ro/trn_rl_repo"):
    if _p not in sys.path:
        sys.path.append(_p)

import concourse.bass as bass
import concourse.mybir as mybir
from concourse import bacc, bass_utils
from concourse.tile import TileContext

B, C, H, W = 2, 64, 64, 64
L = H * W            # 4096
C9 = C * 9           # 576
NSHARD = 4           # Q-column shards per batch
LQ = L // NSHARD     # 1024 query columns per core
NCORES = 8
NQB = LQ // 128      # 8 query blocks of 128
NKT = L // 512       # 8 key column tiles of 512
# contraction chunks over C9=576: rows (start, size)
CHUNKS = [(0, 128), (128, 128), (256, 128), (384, 128), (512, 64)]

EPS = 1e-12

_BASS_CACHE = {}


def _build_bass():
    f32 = mybir.dt.float32
    f32r = mybir.dt.float32r  # same fp32 bits, 4x faster PE streaming
    u32 = mybir.dt.uint32
    # Bacc (not plain Bass): its compile() runs move_matmul_waits_to_ldweights
    # + generate_event_semaphores, which split multi-wait instructions that
    # walrus otherwise rejects ("Too many sync wait commands")
    nc = bacc.Bacc("TRN2")

    kn = nc.dram_tensor("kn", [C9, L], f32r, kind="ExternalInput")
    qn = nc.dram_tensor("qn", [C9, LQ], f32r, kind="ExternalInput")
    # top-8 candidate indices per query column (host re-scores them exactly)
    rarg = nc.dram_tensor("rarg", [128, NQB * 8], u32, kind="ExternalOutput")

    with TileContext(nc) as tc:
        with (
            tc.tile_pool(name="kpool", bufs=1) as kpool,
            tc.tile_pool(name="qpool", bufs=1) as qpool,
            tc.tile_pool(name="rpool", bufs=3) as rpool,
            tc.tile_pool(name="ppool", bufs=2, space="PSUM") as ppool,
            tc.tile_pool(name="mpool", bufs=2) as mpool,
            tc.tile_pool(name="opool", bufs=1) as opool,
        ):
            # persistent input tiles; small tiles so compute can start as
            # soon as the first slices land
            qt = [
                [
                    qpool.tile([p, 128], f32r, name=f"qt{t}_{qb}", tag=f"qt{t}_{qb}")
                    for qb in range(NQB)
                ]
                for t, (r0, p) in enumerate(CHUNKS)
            ]
            kt = [
                [
                    kpool.tile([p, 512], f32r, name=f"kt{t}_{n}", tag=f"kt{t}_{n}")
                    for n in range(NKT)
                ]
                for t, (r0, p) in enumerate(CHUNKS)
            ]
            # DMA issue order ~ consumption order
            for t, (r0, p) in enumerate(CHUNKS):
                nc.sync.dma_start(out=qt[t][0], in_=qn[r0 : r0 + p, 0:128])
            for n in range(NKT):
                for t, (r0, p) in enumerate(CHUNKS):
                    nc.sync.dma_start(
                        out=kt[t][n], in_=kn[r0 : r0 + p, n * 512 : (n + 1) * 512]
                    )
            for qb in range(1, NQB):
                for t, (r0, p) in enumerate(CHUNKS):
                    nc.sync.dma_start(
                        out=qt[t][qb], in_=qn[r0 : r0 + p, qb * 128 : (qb + 1) * 128]
                    )

            outi = opool.tile([128, NQB * 8], u32, name="outi", tag="outi")

            for qb in range(NQB):
                rt = rpool.tile([128, L], f32, name="rt", tag="rt")
                for kh in range(2):
                    ps = ppool.tile([128, 2048], f32, name="ps", tag="ps")
                    for t in range(5):
                        for n in range(4):
                            nc.tensor.matmul(
                                ps[:, n * 512 : (n + 1) * 512],
                                lhsT=qt[t][qb],
                                rhs=kt[t][kh * 4 + n],
                                start=(t == 0),
                                stop=(t == 4),
                            )
                    nc.scalar.copy(out=rt[:, kh * 2048 : (kh + 1) * 2048], in_=ps)
                mx8 = mpool.tile([128, 8], f32, name="mx8", tag="mx8")
                nc.vector.max(out=mx8, in_=rt)
                idx8 = mpool.tile([128, 8], u32, name="idx8", tag="idx8")
                nc.vector.max_index(out=idx8, in_max=mx8, in_values=rt)
                nc.gpsimd.tensor_copy(
                    out=outi[:, qb * 8 : (qb + 1) * 8], in_=idx8
                )

            nc.sync.dma_start(out=rarg[:, :], in_=outi)
    if not nc.is_finalized():
        nc.finalize()
    return nc


def _unfold_ij(x):
    """[B,C,H,W] -> [B, 9*C, H*W] with row = ij*C + c (ij-major order)."""
    b, c, h, w = x.shape
    xp = np.pad(x, ((0, 0), (0, 0), (1, 1), (1, 1)))
    blocks = [
        xp[:, :, i : i + h, j : j + w].reshape(b, c, h * w)
        for i in range(3)
        for j in range(3)
    ]
    return np.concatenate(blocks, axis=1)


def _unfold_torch(x):
    """[B,C,H,W] -> [B, C*9, H*W] in torch F.unfold order (c-major)."""
    b, c, h, w = x.shape
    xp = np.pad(x, ((0, 0), (0, 0), (1, 1), (1, 1)))
    patches = np.stack(
        [xp[:, :, i : i + h, j : j + w] for i in range(3) for j in range(3)],
        axis=2,
    )
    return patches.reshape(b, c * 9, h * w)


def _fold_torch(u, h, w):
    """Inverse layout of _unfold_torch: sum overlapping patches."""
    b, ck, l = u.shape
    c = ck // 9
    p = u.reshape(b, c, 3, 3, h, w)
    out = np.zeros((b, c, h + 2, w + 2), u.dtype)
    for i in range(3):
        for j in range(3):
            out[:, :, i : i + h, j : j + w] += p[:, :, i, j]
    return out[:, :, 1 : 1 + h, 1 : 1 + w]


def _l2n_cols(x):
    """Normalize columns of [B, C9, L] (fp32, eps as in reference)."""
    n = np.sqrt(np.sum(x * x, axis=1, keepdims=True, dtype=np.float32))
    return x / np.maximum(n, EPS)


def _run_device(Kn, Qn, trace=False, trace_cores=None):
    key = "nc"
    if key not in _BASS_CACHE:
        _BASS_CACHE[key] = _build_bass()
    nc = _BASS_CACHE[key]
    in_maps = []
    for ci in range(NCORES):
        b, s = divmod(ci, NSHARD)
        in_maps.append(
            {
                "kn": np.ascontiguousarray(Kn[b]),
                "qn": np.ascontiguousarray(Qn[b][:, s * LQ : (s + 1) * LQ]),
            }
        )
    res = bass_utils.run_bass_kernel_spmd(
        nc,
        in_maps,
        core_ids=list(range(NCORES)),
        trace=trace,
        trace_cores=trace_cores,
    )
    return res


def kernel(V, K, Q, _trace=False, _trace_cores=None, _return_results=False):
    V = np.asarray(V, dtype=np.float32)
    K = np.asarray(K, dtype=np.float32)
    Q = np.asarray(Q, dtype=np.float32)

    Kn = _l2n_cols(_unfold_ij(K))
    Qn = _l2n_cols(_unfold_ij(Q))

    res = _run_device(Kn, Qn, trace=_trace, trace_cores=_trace_cores)

    # device returns top-8 candidate k per query (under fp32r matmul scores);
    # re-score the 8 candidates exactly in fp32 to pick the true argmax/max
    cand = np.empty((B, L, 8), np.int64)
    for ci in range(NCORES):
        b, s = divmod(ci, NSHARD)
        out = np.asarray(res.results[ci]["rarg"]).astype(np.int64)
        # out[p, qb*8+j] -> local q index qb*128 + p
        cand[b, s * LQ : (s + 1) * LQ] = (
            out.reshape(128, NQB, 8).transpose(1, 0, 2).reshape(LQ, 8)
        )
    np.clip(cand, 0, L - 1, out=cand)  # -1 "unmatched" sentinel -> harmless dup

    rstar = np.empty((B, L), np.float32)
    rarg = np.empty((B, L), np.int64)
    for b in range(B):
        kc = Kn[b][:, cand[b].reshape(-1)].reshape(C9, L, 8)
        scores = np.einsum("cqj,cq->qj", kc, Qn[b], dtype=np.float64)
        maxv = scores.max(axis=1, keepdims=True)
        kmask = np.where(scores == maxv, cand[b], 1 << 40)
        rarg[b] = kmask.min(axis=1)  # first occurrence on ties, like argmax
        rstar[b] = maxv[:, 0].astype(np.float32)

    V_unf = _unfold_torch(V)
    T_unf = np.take_along_axis(V_unf, rarg[:, None, :], axis=2)
    T = (_fold_torch(T_unf, H, W) / 9.0).astype(np.float32)
    S = rstar.reshape(B, 1, H, W)

    if _return_results:
        return (S, T), res
    return (S, T)


# revision 14
# speedup vs baseline: 2.6753x; 1.1011x over previous
"""DINet retrieval-knn kernel for 8 trn2 NeuronCores.

Math (see reference): for each query patch q (3x3xC neighborhood of Q),
find k* = argmax_k cos(K_patch_k, Q_patch_q) over all 4096 key patches,
output S = max cosine value, T = fold(V_patch_gather(k*)) / 9.

Device strategy (per sharding hint): data-parallel over batch B (=2),
sequence-parallel over Q columns (4 shards of 1024) -> 8 cores. Each core
computes its full [Lk=4096, Lq=1024] correlation block with the tensor
engine (contraction C*9=576 in fp32), and a fused
copy+max (tensor_tensor_reduce) plus max_index pass gives max/argmax over
the full K axis per query. Host does layout prep (unfold, l2-normalize)
and the final V-gather + fold.
"""

import sys

import numpy as np

for _p in ("/opt/trn_rl_repo", "/root/.axon_site/_ro/trn_rl_repo"):
    if _p not in sys.path:
        sys.path.append(_p)

import concourse.bass as bass
import concourse.mybir as mybir
from concourse import bacc, bass_utils
from concourse.tile import TileContext

B, C, H, W = 2, 64, 64, 64
L = H * W            # 4096
C9 = C * 9           # 576
NSHARD = 4           # Q-column shards per batch
LQ = L // NSHARD     # 1024 query columns per core
NCORES = 8
NQB = LQ // 128      # 8 query blocks of 128
NKT = L // 512       # 8 key column tiles of 512
# contraction chunks over C9=576: rows (start, size)
CHUNKS = [(0, 128), (128, 128), (256, 128), (384, 128), (512, 64)]

EPS = 1e-12

_BASS_CACHE = {}


def _build_bass():
    f32 = mybir.dt.float32
    bf16 = mybir.dt.bfloat16  # full-rate PE + FWL weight loads + half DMA
    u32 = mybir.dt.uint32
    # Bacc (not plain Bass): its compile() runs move_matmul_waits_to_ldweights
    # + generate_event_semaphores, which split multi-wait instructions that
    # walrus otherwise rejects ("Too many sync wait commands")
    nc = bacc.Bacc("TRN2")

    kn = nc.dram_tensor("kn", [C9, L], bf16, kind="ExternalInput")
    qn = nc.dram_tensor("qn", [C9, LQ], bf16, kind="ExternalInput")
    # top-8 candidate indices per (query column, k-half); host re-scores
    # the 16 candidates exactly in fp32
    rarg = nc.dram_tensor("rarg", [128, NQB * 16], u32, kind="ExternalOutput")

    with TileContext(nc) as tc:
        with (
            tc.tile_pool(name="kpool", bufs=1) as kpool,
            tc.tile_pool(name="qpool", bufs=1) as qpool,
            tc.tile_pool(name="rpool", bufs=3) as rpool,
            tc.tile_pool(name="ppool", bufs=2, space="PSUM") as ppool,
            tc.tile_pool(name="mpool", bufs=2) as mpool,
            tc.tile_pool(name="opool", bufs=1) as opool,
        ):
            # persistent input tiles; small tiles so compute can start as
            # soon as the first slices land
            qt = [
                [
                    qpool.tile([p, 128], bf16, name=f"qt{t}_{qb}", tag=f"qt{t}_{qb}")
                    for qb in range(NQB)
                ]
                for t, (r0, p) in enumerate(CHUNKS)
            ]
            kt = [
                [
                    kpool.tile([p, 512], bf16, name=f"kt{t}_{n}", tag=f"kt{t}_{n}")
                    for n in range(NKT)
                ]
                for t, (r0, p) in enumerate(CHUNKS)
            ]
            # DMA issue order ~ consumption order
            for t, (r0, p) in enumerate(CHUNKS):
                nc.sync.dma_start(out=qt[t][0], in_=qn[r0 : r0 + p, 0:128])
            for n in range(NKT):
                for t, (r0, p) in enumerate(CHUNKS):
                    nc.sync.dma_start(
                        out=kt[t][n], in_=kn[r0 : r0 + p, n * 512 : (n + 1) * 512]
                    )
            for qb in range(1, NQB):
                for t, (r0, p) in enumerate(CHUNKS):
                    nc.sync.dma_start(
                        out=qt[t][qb], in_=qn[r0 : r0 + p, qb * 128 : (qb + 1) * 128]
                    )

            outi = opool.tile([128, NQB * 16], u32, name="outi", tag="outi")

            for qb in range(NQB):
                for kh in range(2):
                    ps = ppool.tile([128, 2048], f32, name="ps", tag="ps")
                    for t in range(5):
                        for n in range(4):
                            nc.tensor.matmul(
                                ps[:, n * 512 : (n + 1) * 512],
                                lhsT=qt[t][qb],
                                rhs=kt[t][kh * 4 + n],
                                start=(t == 0),
                                stop=(t == 4),
                            )
                    # downcast to bf16: halves the DVE scan cost (2x mode)
                    rt = rpool.tile([128, 2048], bf16, name="rt", tag="rt")
                    nc.scalar.copy(out=rt, in_=ps)
                    mx8 = mpool.tile([128, 8], bf16, name="mx8", tag="mx8")
                    nc.vector.max(out=mx8, in_=rt)
                    idx8 = mpool.tile([128, 8], u32, name="idx8", tag="idx8")
                    nc.vector.max_index(out=idx8, in_max=mx8, in_values=rt)
                    nc.gpsimd.tensor_copy(
                        out=outi[:, (qb * 2 + kh) * 8 : (qb * 2 + kh + 1) * 8],
                        in_=idx8,
                    )

            nc.sync.dma_start(out=rarg[:, :], in_=outi)
    if not nc.is_finalized():
        nc.finalize()
    return nc


def _unfold_ij(x):
    """[B,C,H,W] -> [B, 9*C, H*W] with row = ij*C + c (ij-major order)."""
    b, c, h, w = x.shape
    xp = np.pad(x, ((0, 0), (0, 0), (1, 1), (1, 1)))
    blocks = [
        xp[:, :, i : i + h, j : j + w].reshape(b, c, h * w)
        for i in range(3)
        for j in range(3)
    ]
    return np.concatenate(blocks, axis=1)


def _unfold_torch(x):
    """[B,C,H,W] -> [B, C*9, H*W] in torch F.unfold order (c-major)."""
    b, c, h, w = x.shape
    xp = np.pad(x, ((0, 0), (0, 0), (1, 1), (1, 1)))
    patches = np.stack(
        [xp[:, :, i : i + h, j : j + w] for i in range(3) for j in range(3)],
        axis=2,
    )
    return patches.reshape(b, c * 9, h * w)


def _fold_torch(u, h, w):
    """Inverse layout of _unfold_torch: sum overlapping patches."""
    b, ck, l = u.shape
    c = ck // 9
    p = u.reshape(b, c, 3, 3, h, w)
    out = np.zeros((b, c, h + 2, w + 2), u.dtype)
    for i in range(3):
        for j in range(3):
            out[:, :, i : i + h, j : j + w] += p[:, :, i, j]
    return out[:, :, 1 : 1 + h, 1 : 1 + w]


def _l2n_cols(x):
    """Normalize columns of [B, C9, L] (fp32, eps as in reference)."""
    n = np.sqrt(np.sum(x * x, axis=1, keepdims=True, dtype=np.float32))
    return x / np.maximum(n, EPS)


def _run_device(Kn, Qn, trace=False, trace_cores=None):
    import ml_dtypes

    key = "nc"
    if key not in _BASS_CACHE:
        _BASS_CACHE[key] = _build_bass()
    nc = _BASS_CACHE[key]
    bf = ml_dtypes.bfloat16
    in_maps = []
    for ci in range(NCORES):
        b, s = divmod(ci, NSHARD)
        in_maps.append(
            {
                "kn": np.ascontiguousarray(Kn[b].astype(bf)),
                "qn": np.ascontiguousarray(Qn[b][:, s * LQ : (s + 1) * LQ].astype(bf)),
            }
        )
    res = bass_utils.run_bass_kernel_spmd(
        nc,
        in_maps,
        core_ids=list(range(NCORES)),
        trace=trace,
        trace_cores=trace_cores,
    )
    return res


def kernel(V, K, Q, _trace=False, _trace_cores=None, _return_results=False):
    V = np.asarray(V, dtype=np.float32)
    K = np.asarray(K, dtype=np.float32)
    Q = np.asarray(Q, dtype=np.float32)

    Kn = _l2n_cols(_unfold_ij(K))
    Qn = _l2n_cols(_unfold_ij(Q))

    res = _run_device(Kn, Qn, trace=_trace, trace_cores=_trace_cores)

    # device returns top-8 candidate k per (query, 2048-wide k-half) under
    # bf16 matmul scores; re-score the 16 candidates exactly in fp32
    cand = np.empty((B, L, 16), np.int64)
    for ci in range(NCORES):
        b, s = divmod(ci, NSHARD)
        out = np.asarray(res.results[ci]["rarg"]).astype(np.int64)
        # out[p, (qb*2+kh)*8+j] -> local q index qb*128 + p, k = kh*2048 + idx
        c = out.reshape(128, NQB, 2, 8)
        c = np.clip(c, 0, 2047) + np.arange(2)[None, None, :, None] * 2048
        cand[b, s * LQ : (s + 1) * LQ] = (
            c.reshape(128, NQB, 16).transpose(1, 0, 2).reshape(LQ, 16)
        )

    rstar = np.empty((B, L), np.float32)
    rarg = np.empty((B, L), np.int64)
    for b in range(B):
        kc = Kn[b][:, cand[b].reshape(-1)].reshape(C9, L, 16)
        scores = np.einsum("cqj,cq->qj", kc, Qn[b], dtype=np.float64)
        maxv = scores.max(axis=1, keepdims=True)
        kmask = np.where(scores == maxv, cand[b], 1 << 40)
        rarg[b] = kmask.min(axis=1)  # first occurrence on ties, like argmax
        rstar[b] = maxv[:, 0].astype(np.float32)

    V_unf = _unfold_torch(V)
    T_unf = np.take_along_axis(V_unf, rarg[:, None, :], axis=2)
    T = (_fold_torch(T_unf, H, W) / 9.0).astype(np.float32)
    S = rstar.reshape(B, 1, H, W)

    if _return_results:
        return (S, T), res
    return (S, T)


# revision 15
# speedup vs baseline: 3.0701x; 1.1476x over previous
"""DINet retrieval-knn kernel for 8 trn2 NeuronCores.

Math (see reference): for each query patch q (3x3xC neighborhood of Q),
find k* = argmax_k cos(K_patch_k, Q_patch_q) over all 4096 key patches,
output S = max cosine value, T = fold(V_patch_gather(k*)) / 9.

Device strategy (per sharding hint): data-parallel over batch B (=2),
sequence-parallel over Q columns (4 shards of 1024) -> 8 cores. Each core
computes its full [Lk=4096, Lq=1024] correlation block with the tensor
engine (contraction C*9=576 in fp32), and a fused
copy+max (tensor_tensor_reduce) plus max_index pass gives max/argmax over
the full K axis per query. Host does layout prep (unfold, l2-normalize)
and the final V-gather + fold.
"""

import sys

import numpy as np

for _p in ("/opt/trn_rl_repo", "/root/.axon_site/_ro/trn_rl_repo"):
    if _p not in sys.path:
        sys.path.append(_p)

import concourse.bass as bass
import concourse.mybir as mybir
from concourse import bacc, bass_utils
from concourse.tile import TileContext

B, C, H, W = 2, 64, 64, 64
L = H * W            # 4096
C9 = C * 9           # 576
NSHARD = 4           # Q-column shards per batch
LQ = L // NSHARD     # 1024 query columns per core
NCORES = 8
NQB = LQ // 128      # 8 query blocks of 128
NKT = L // 512       # 8 key column tiles of 512
# contraction chunks over C9=576: rows (start, size)
CHUNKS = [(0, 128), (128, 128), (256, 128), (384, 128), (512, 64)]

EPS = 1e-12

_BASS_CACHE = {}


def _build_bass():
    f32 = mybir.dt.float32
    bf16 = mybir.dt.bfloat16  # full-rate PE + FWL weight loads + half DMA
    u32 = mybir.dt.uint32
    # Bacc (not plain Bass): its compile() runs move_matmul_waits_to_ldweights
    # + generate_event_semaphores, which split multi-wait instructions that
    # walrus otherwise rejects ("Too many sync wait commands")
    nc = bacc.Bacc("TRN2")

    kn = nc.dram_tensor("kn", [C9, L], bf16, kind="ExternalInput")
    qn = nc.dram_tensor("qn", [C9, LQ], bf16, kind="ExternalInput")
    # top-8 candidate indices per (query column, k-half); host re-scores
    # the 16 candidates exactly in fp32
    rarg = nc.dram_tensor("rarg", [128, NQB * 16], u32, kind="ExternalOutput")

    with TileContext(nc) as tc:
        with (
            tc.tile_pool(name="kpool", bufs=1) as kpool,
            tc.tile_pool(name="qpool", bufs=1) as qpool,
            tc.tile_pool(name="rpool", bufs=3) as rpool,
            tc.tile_pool(name="ppool", bufs=2, space="PSUM") as ppool,
            tc.tile_pool(name="mpool", bufs=2) as mpool,
            tc.tile_pool(name="opool", bufs=1) as opool,
        ):
            # persistent input tiles; one DMA per (chunk, k-half) keeps the
            # issue cost on the Sync sequencer low while still letting the
            # first matmul group start after ~1/2 of the K data has landed
            qt = [
                qpool.tile([p, LQ], bf16, name=f"qt{t}", tag=f"qt{t}")
                for t, (r0, p) in enumerate(CHUNKS)
            ]
            kt = [
                [
                    kpool.tile([p, 2048], bf16, name=f"kt{t}_{kh}", tag=f"kt{t}_{kh}")
                    for kh in range(2)
                ]
                for t, (r0, p) in enumerate(CHUNKS)
            ]
            # DMA issue order ~ consumption order
            for t, (r0, p) in enumerate(CHUNKS):
                nc.sync.dma_start(out=qt[t], in_=qn[r0 : r0 + p, :])
            for kh in range(2):
                for t, (r0, p) in enumerate(CHUNKS):
                    nc.sync.dma_start(
                        out=kt[t][kh], in_=kn[r0 : r0 + p, kh * 2048 : (kh + 1) * 2048]
                    )

            outi = opool.tile([128, NQB * 16], u32, name="outi", tag="outi")

            for qb in range(NQB):
                for kh in range(2):
                    ps = ppool.tile([128, 2048], f32, name="ps", tag="ps")
                    for t in range(5):
                        for n in range(4):
                            nc.tensor.matmul(
                                ps[:, n * 512 : (n + 1) * 512],
                                lhsT=qt[t][:, qb * 128 : (qb + 1) * 128],
                                rhs=kt[t][kh][:, n * 512 : (n + 1) * 512],
                                start=(t == 0),
                                stop=(t == 4),
                            )
                    # downcast to bf16: halves the DVE scan cost (2x mode)
                    rt = rpool.tile([128, 2048], bf16, name="rt", tag="rt")
                    nc.scalar.copy(out=rt, in_=ps)
                    mx8 = mpool.tile([128, 8], bf16, name="mx8", tag="mx8")
                    nc.vector.max(out=mx8, in_=rt)
                    idx8 = mpool.tile([128, 8], u32, name="idx8", tag="idx8")
                    nc.vector.max_index(out=idx8, in_max=mx8, in_values=rt)
                    nc.gpsimd.tensor_copy(
                        out=outi[:, (qb * 2 + kh) * 8 : (qb * 2 + kh + 1) * 8],
                        in_=idx8,
                    )

            nc.sync.dma_start(out=rarg[:, :], in_=outi)
    if not nc.is_finalized():
        nc.finalize()
    return nc


def _unfold_ij(x):
    """[B,C,H,W] -> [B, 9*C, H*W] with row = ij*C + c (ij-major order)."""
    b, c, h, w = x.shape
    xp = np.pad(x, ((0, 0), (0, 0), (1, 1), (1, 1)))
    blocks = [
        xp[:, :, i : i + h, j : j + w].reshape(b, c, h * w)
        for i in range(3)
        for j in range(3)
    ]
    return np.concatenate(blocks, axis=1)


def _unfold_torch(x):
    """[B,C,H,W] -> [B, C*9, H*W] in torch F.unfold order (c-major)."""
    b, c, h, w = x.shape
    xp = np.pad(x, ((0, 0), (0, 0), (1, 1), (1, 1)))
    patches = np.stack(
        [xp[:, :, i : i + h, j : j + w] for i in range(3) for j in range(3)],
        axis=2,
    )
    return patches.reshape(b, c * 9, h * w)


def _fold_torch(u, h, w):
    """Inverse layout of _unfold_torch: sum overlapping patches."""
    b, ck, l = u.shape
    c = ck // 9
    p = u.reshape(b, c, 3, 3, h, w)
    out = np.zeros((b, c, h + 2, w + 2), u.dtype)
    for i in range(3):
        for j in range(3):
            out[:, :, i : i + h, j : j + w] += p[:, :, i, j]
    return out[:, :, 1 : 1 + h, 1 : 1 + w]


def _l2n_cols(x):
    """Normalize columns of [B, C9, L] (fp32, eps as in reference)."""
    n = np.sqrt(np.sum(x * x, axis=1, keepdims=True, dtype=np.float32))
    return x / np.maximum(n, EPS)


def _run_device(Kn, Qn, trace=False, trace_cores=None):
    import ml_dtypes

    key = "nc"
    if key not in _BASS_CACHE:
        _BASS_CACHE[key] = _build_bass()
    nc = _BASS_CACHE[key]
    bf = ml_dtypes.bfloat16
    in_maps = []
    for ci in range(NCORES):
        b, s = divmod(ci, NSHARD)
        in_maps.append(
            {
                "kn": np.ascontiguousarray(Kn[b].astype(bf)),
                "qn": np.ascontiguousarray(Qn[b][:, s * LQ : (s + 1) * LQ].astype(bf)),
            }
        )
    res = bass_utils.run_bass_kernel_spmd(
        nc,
        in_maps,
        core_ids=list(range(NCORES)),
        trace=trace,
        trace_cores=trace_cores,
    )
    return res


def kernel(V, K, Q, _trace=False, _trace_cores=None, _return_results=False):
    V = np.asarray(V, dtype=np.float32)
    K = np.asarray(K, dtype=np.float32)
    Q = np.asarray(Q, dtype=np.float32)

    Kn = _l2n_cols(_unfold_ij(K))
    Qn = _l2n_cols(_unfold_ij(Q))

    res = _run_device(Kn, Qn, trace=_trace, trace_cores=_trace_cores)

    # device returns top-8 candidate k per (query, 2048-wide k-half) under
    # bf16 matmul scores; re-score the 16 candidates exactly in fp32
    cand = np.empty((B, L, 16), np.int64)
    for ci in range(NCORES):
        b, s = divmod(ci, NSHARD)
        out = np.asarray(res.results[ci]["rarg"]).astype(np.int64)
        # out[p, (qb*2+kh)*8+j] -> local q index qb*128 + p, k = kh*2048 + idx
        c = out.reshape(128, NQB, 2, 8)
        c = np.clip(c, 0, 2047) + np.arange(2)[None, None, :, None] * 2048
        cand[b, s * LQ : (s + 1) * LQ] = (
            c.reshape(128, NQB, 16).transpose(1, 0, 2).reshape(LQ, 16)
        )

    rstar = np.empty((B, L), np.float32)
    rarg = np.empty((B, L), np.int64)
    for b in range(B):
        kc = Kn[b][:, cand[b].reshape(-1)].reshape(C9, L, 16)
        scores = np.einsum("cqj,cq->qj", kc, Qn[b], dtype=np.float64)
        maxv = scores.max(axis=1, keepdims=True)
        kmask = np.where(scores == maxv, cand[b], 1 << 40)
        rarg[b] = kmask.min(axis=1)  # first occurrence on ties, like argmax
        rstar[b] = maxv[:, 0].astype(np.float32)

    V_unf = _unfold_torch(V)
    T_unf = np.take_along_axis(V_unf, rarg[:, None, :], axis=2)
    T = (_fold_torch(T_unf, H, W) / 9.0).astype(np.float32)
    S = rstar.reshape(B, 1, H, W)

    if _return_results:
        return (S, T), res
    return (S, T)
